# revision 1
# baseline (speedup 1.0000x reference)
"""Trainium2 Bass kernel for DanceDecoder: 2-layer autoregressive LSTM.

B=8192, T=60, HID=512, OUT=51, LAT=64.  Data-parallel over 8 cores
(1024 batch rows each).  Feature-major layout: features on SBUF
partitions, batch in the free dimension (2 blocks of 512 columns).
Matmuls run in float32r (single-pass fp32, ~TF32 precision) with fp32
PSUM accumulation; the c state and all biases stay fp32.
"""
import sys
sys.path.insert(0, "/opt/trn_rl_repo")

import numpy as np
import concourse.bacc as bacc
import concourse.mybir as mybir
import concourse.tile as tile
from concourse.bass_utils import run_bass_kernel_spmd

HID = 512
OUT = 51
LAT = 64
T = 60
B = 8192
NCORES = 8
BC = B // NCORES          # 1024 batch columns per core
NBLK = 2                  # column blocks per core
NB = BC // NBLK           # 512 columns per block
KH = HID // 128           # 4 hidden chunks

F32 = mybir.dt.float32
F32R = mybir.dt.float32r
AF = mybir.ActivationFunctionType
OP = mybir.AluOpType

_cached = {}


def build_module(unroll_T=T, debug_taps=False, repeat=1):
    nc = bacc.Bacc("TRN2", target_bir_lowering=False, debug=False)

    # ---- DRAM I/O (per core) ----
    zT = nc.dram_tensor("zT", [LAT, BC], F32R, kind="ExternalInput")
    x0 = nc.dram_tensor("x0", [OUT, BC], F32R, kind="ExternalInput")
    wih0T = nc.dram_tensor("wih0T", [OUT, 4 * HID], F32R, kind="ExternalInput")
    whh0T = nc.dram_tensor("whh0T", [HID, 4 * HID], F32R, kind="ExternalInput")
    w1T = nc.dram_tensor("w1T", [2 * HID, 4 * HID], F32R, kind="ExternalInput")
    foutT = nc.dram_tensor("foutT", [HID, OUT], F32R, kind="ExternalInput")
    finitT = nc.dram_tensor("finitT", [LAT, 2 * HID], F32R, kind="ExternalInput")
    b0c = nc.dram_tensor("b0c", [128, 16], F32, kind="ExternalInput")
    b1c = nc.dram_tensor("b1c", [128, 16], F32, kind="ExternalInput")
    binitc = nc.dram_tensor("binitc", [128, 2 * KH], F32, kind="ExternalInput")
    boutc = nc.dram_tensor("boutc", [OUT, 1], F32, kind="ExternalInput")
    frames = nc.dram_tensor("frames", [unroll_T, OUT, BC], F32,
                            kind="ExternalOutput")

    with tile.TileContext(nc) as tc:
        with (
            tc.tile_pool(name="wpool", bufs=1) as wp,
            tc.tile_pool(name="spool", bufs=1) as sp,
            tc.tile_pool(name="tmp", bufs=2) as tp,
            tc.tile_pool(name="psum", bufs=2, space="PSUM") as pp,
        ):
            # ---- persistent SBUF tiles ----
            w_ih0 = wp.tile([OUT, 4 * HID], F32R, tag="w_ih0")
            w_hh0 = wp.tile([128, KH, 4 * HID], F32R, tag="w_hh0")
            w_1 = wp.tile([128, 2 * KH, 4 * HID], F32R, tag="w_1")
            w_out = wp.tile([128, KH, OUT], F32R, tag="w_out")
            w_init = wp.tile([LAT, 2 * HID], F32R, tag="w_init")
            bias0 = wp.tile([128, 16], F32, tag="bias0")
            bias1 = wp.tile([128, 16], F32, tag="bias1")
            biasi = wp.tile([128, 2 * KH], F32, tag="biasi")
            biaso = wp.tile([OUT, 1], F32, tag="biaso")
            z_t = wp.tile([LAT, BC], F32R, tag="z_t")

            h1 = sp.tile([128, KH, BC], F32R, tag="h1")
            c1 = sp.tile([128, KH, BC], F32, tag="c1")
            h2 = sp.tile([128, KH, BC], F32R, tag="h2")
            c2 = sp.tile([128, KH, BC], F32, tag="c2")
            x = sp.tile([OUT, BC], F32R, tag="x")

            # ---- load everything ----
            nc.sync.dma_start(w_ih0[:], wih0T[:])
            for j in range(KH):
                nc.sync.dma_start(w_hh0[:, j, :], whh0T[j * 128:(j + 1) * 128, :])
                nc.sync.dma_start(w_out[:, j, :], foutT[j * 128:(j + 1) * 128, :])
            for j in range(2 * KH):
                nc.sync.dma_start(w_1[:, j, :], w1T[j * 128:(j + 1) * 128, :])
            nc.sync.dma_start(w_init[:], finitT[:])
            nc.sync.dma_start(bias0[:], b0c[:])
            nc.sync.dma_start(bias1[:], b1c[:])
            nc.sync.dma_start(biasi[:], binitc[:])
            nc.sync.dma_start(biaso[:], boutc[:])
            nc.sync.dma_start(z_t[:], zT[:])
            nc.sync.dma_start(x[:], x0[:])

            # ---- init: h0/c0 = fc_init(z), replicated into both layers ----
            def init_pass(rep):
                if rep > 0:
                    nc.sync.dma_start(x[:], x0[:])
                for b in range(NBLK):
                    s = b * NB
                    for m in range(2 * KH):
                        acc = pp.tile([128, NB], F32, tag=f"g{m % 4}")
                        nc.tensor.matmul(acc[:],
                                         w_init[:, m * 128:(m + 1) * 128],
                                         z_t[:, s:s + NB],
                                         start=True, stop=True)
                        if m < KH:
                            dsts = (h1[:, m, s:s + NB], h2[:, m, s:s + NB])
                        else:
                            dsts = (c1[:, m - KH, s:s + NB],
                                    c2[:, m - KH, s:s + NB])
                        nc.vector.tensor_scalar(dsts[0], acc[:],
                                                biasi[:, m:m + 1],
                                                None, OP.add)
                        nc.vector.tensor_copy(dsts[1], dsts[0])

            # ---- one LSTM cell update for (layer, block, chunk) ----
            # Gate results for the new h go to a per-chunk staging tile
            # (hnew); the caller commits them into the state tile only after
            # every matmul of the phase has been traced, so all units read
            # the previous step's h.
            def cell(b, k, c_st, bias, in_mms):
                """in_mms: list of (lhsT_ap, rhs_ap) contraction terms."""
                s = b * NB
                P = {}
                for g in ("i", "g", "f", "o"):
                    gi = {"i": 0, "f": 1, "g": 2, "o": 3}[g]
                    acc = pp.tile([128, NB], F32, tag=f"g{gi}")
                    P[g] = acc
                    col = gi * HID + k * 128
                    n = len(in_mms)
                    for t_, (lhsT, rhs) in enumerate(in_mms):
                        nc.tensor.matmul(acc[:], lhsT[:, col:col + 128], rhs,
                                         start=(t_ == 0), stop=(t_ == n - 1))
                # activations: i/f/o sigmoid in place on PSUM; tanh(g) lands
                # in the SBUF temp (DVE can read at most one PSUM operand).
                ig = tp.tile([128, NB], F32, tag="ig")
                hn = tp.tile([128, NB], F32R, tag=f"hnew{k}")
                nc.scalar.activation(P["i"][:], P["i"][:], AF.Sigmoid,
                                     bias=bias[:, k:k + 1])
                nc.scalar.activation(ig[:], P["g"][:], AF.Tanh,
                                     bias=bias[:, 8 + k:8 + k + 1])
                nc.scalar.activation(P["f"][:], P["f"][:], AF.Sigmoid,
                                     bias=bias[:, 4 + k:4 + k + 1])
                nc.scalar.activation(P["o"][:], P["o"][:], AF.Sigmoid,
                                     bias=bias[:, 12 + k:12 + k + 1])
                cs = c_st[:, k, s:s + NB]
                nc.vector.tensor_tensor(ig[:], P["i"][:], ig[:], OP.mult)
                nc.vector.tensor_tensor(cs, P["f"][:], cs, OP.mult)
                nc.vector.tensor_tensor(cs, cs, ig[:], OP.add)
                nc.scalar.activation(hn[:], cs, AF.Tanh)
                nc.vector.tensor_tensor(hn[:], P["o"][:], hn[:], OP.mult)
                return hn

            # ---- the 60 autoregressive steps ----
            for _rep in range(repeat):
              init_pass(_rep)
              if debug_taps and _rep == 0:
                d_h1i = nc.dram_tensor("d_h1i", [128, KH, BC], F32R,
                                       kind="ExternalOutput")
                d_c1i = nc.dram_tensor("d_c1i", [128, KH, BC], F32,
                                       kind="ExternalOutput")
                nc.sync.dma_start(d_h1i[:], h1[:])
                nc.sync.dma_start(d_c1i[:], c1[:])
              for t in range(unroll_T):
                for b in range(NBLK):
                    s = b * NB
                    hns = []
                    for k in range(KH):
                        mms = [(w_hh0[:, j, :], h1[:, j, s:s + NB])
                               for j in range(KH)]
                        mms.append((w_ih0[:], x[:, s:s + NB]))
                        hns.append(cell(b, k, c1, bias0, mms))
                    for k in range(KH):
                        nc.vector.tensor_copy(h1[:, k, s:s + NB], hns[k][:])
                if debug_taps and t == 0:
                    d_h1s = nc.dram_tensor("d_h1s", [128, KH, BC], F32R,
                                           kind="ExternalOutput")
                    d_c1s = nc.dram_tensor("d_c1s", [128, KH, BC], F32,
                                           kind="ExternalOutput")
                    nc.sync.dma_start(d_h1s[:], h1[:])
                    nc.sync.dma_start(d_c1s[:], c1[:])
                for b in range(NBLK):
                    s = b * NB
                    hns = []
                    for k in range(KH):
                        mms = [(w_1[:, j, :], h1[:, j, s:s + NB])
                               for j in range(KH)]
                        mms += [(w_1[:, KH + j, :], h2[:, j, s:s + NB])
                                for j in range(KH)]
                        hns.append(cell(b, k, c2, bias1, mms))
                    for k in range(KH):
                        nc.vector.tensor_copy(h2[:, k, s:s + NB], hns[k][:])
                for b in range(NBLK):
                    s = b * NB
                    acc = pp.tile([OUT, NB], F32, tag="g0")
                    for j in range(KH):
                        nc.tensor.matmul(acc[:], w_out[:, j, :],
                                         h2[:, j, s:s + NB],
                                         start=(j == 0), stop=(j == KH - 1))
                    nc.vector.tensor_scalar(x[:, s:s + NB], acc[:], biaso[:],
                                            None, OP.add)
                    nc.sync.dma_start(frames[t, :, s:s + NB],
                                      x[:, s:s + NB].bitcast(F32))

    nc.compile()
    return nc


def _prep_inputs(z, start_token, fc_init_w, fc_init_b,
                 w_ih0, w_hh0, b_ih0, b_hh0,
                 w_ih1, w_hh1, b_ih1, b_hh1,
                 fc_out_w, fc_out_b):
    f32 = np.float32
    common = {
        "wih0T": np.ascontiguousarray(w_ih0.T, dtype=f32),
        "whh0T": np.ascontiguousarray(w_hh0.T, dtype=f32),
        "w1T": np.ascontiguousarray(
            np.concatenate([w_ih1.T, w_hh1.T], axis=0), dtype=f32),
        "foutT": np.ascontiguousarray(fc_out_w.T, dtype=f32),
        "finitT": np.ascontiguousarray(fc_init_w.T, dtype=f32),
        "b0c": np.ascontiguousarray(
            (b_ih0 + b_hh0).reshape(4, 4, 128).transpose(2, 0, 1)
            .reshape(128, 16), dtype=f32),
        "b1c": np.ascontiguousarray(
            (b_ih1 + b_hh1).reshape(4, 4, 128).transpose(2, 0, 1)
            .reshape(128, 16), dtype=f32),
        "binitc": np.ascontiguousarray(
            fc_init_b.reshape(2 * KH, 128).T, dtype=f32),
        "boutc": np.ascontiguousarray(fc_out_b[:, None], dtype=f32),
        "x0": np.ascontiguousarray(
            np.broadcast_to(start_token[:, None], (OUT, BC)), dtype=f32),
    }
    in_maps = []
    for c in range(NCORES):
        m = dict(common)
        m["zT"] = np.ascontiguousarray(
            z[c * BC:(c + 1) * BC].T, dtype=f32)
        in_maps.append(m)
    return in_maps


def kernel(**inputs):
    if "nc" not in _cached:
        _cached["nc"] = build_module()
    nc = _cached["nc"]
    in_maps = _prep_inputs(**inputs)
    res = run_bass_kernel_spmd(nc, in_maps, list(range(NCORES)))
    # frames per core: [T, OUT, BC] -> full [B, T, OUT]
    out = np.stack([res.results[c]["frames"] for c in range(NCORES)])
    return np.ascontiguousarray(
        out.transpose(0, 3, 1, 2).reshape(B, T, OUT))

